# revision 3
# baseline (speedup 1.0000x reference)
"""Trainium2 Bass kernel: per-head (head_dim=128) Walsh-Hadamard transform.

Full input  : value [16384, 4096] f32  (= [tokens, 32 heads * 128])
Full output : same shape; out[t, h*128:(h+1)*128] = (H_128 @ v) / sqrt(128)

Strategy (pure data parallel over tokens, 8 cores, 2048 tokens each):
  - DMA in tiles of [128 tokens, 4096] (contiguous 16KB per partition).
  - Per 128x128 head block B:  Z = B @ H  needs contraction over the free
    axis, so: PE "transpose" matmul #1: B^T = matmul(lhsT=B, rhs=I,
    is_transpose=True)  -> PSUM;  DVE copies PSUM->SBUF;  PE "transpose"
    matmul #2: Z = (B^T)^T @ H = matmul(lhsT=B^T, rhs=H, is_transpose=True)
    -> PSUM (fp32 transpose-mode runs at 2 cyc/row vs 4 for plain fp32 mm).
  - ScalarE activation(Copy, scale=1/sqrt(128)) moves Z PSUM->SBUF.
  - DMA out [128, 4096] tiles.
"""

import math

import numpy as np

import concourse.bass as bass  # noqa: F401  (AP helpers)
import concourse.mybir as mybir
import concourse.tile as tile
from concourse import bacc
from concourse.bass_utils import run_bass_kernel_spmd

HEAD_DIM = 128
N_CORES = 8
TOKENS = 16384
HIDDEN = 4096
P = 128  # partitions / tile token rows


def _hadamard(n: int) -> np.ndarray:
    h = np.array([[1.0]], dtype=np.float64)
    while h.shape[0] < n:
        h = np.block([[h, h], [h, -h]])
    return h


def build_nc(tok_per_core: int = TOKENS // N_CORES, hidden: int = HIDDEN,
             group_heads: int = 4):
    """Build the per-core Bass program. group_heads 128-wide head blocks are
    batched into one PSUM bank ([128, group_heads*128] f32)."""
    assert tok_per_core % P == 0 and hidden % (group_heads * HEAD_DIM) == 0
    gw = group_heads * HEAD_DIM  # group width in columns
    n_tiles = tok_per_core // P
    n_groups = hidden // gw
    scale = float(np.float32(1.0 / math.sqrt(HEAD_DIM)))

    nc = bacc.Bacc("TRN2", target_bir_lowering=False)
    x = nc.dram_tensor("x", [tok_per_core, hidden], mybir.dt.float32,
                       kind="ExternalInput")
    out = nc.dram_tensor("out", [tok_per_core, hidden], mybir.dt.float32,
                         kind="ExternalOutput")
    hm = nc.inline_tensor(_hadamard(HEAD_DIM).astype(np.float32), "hm")
    ident = nc.inline_tensor(np.eye(HEAD_DIM, dtype=np.float32), "ident")

    with tile.TileContext(nc) as tc:
        with (
            tc.tile_pool(name="consts", bufs=1) as cpool,
            tc.tile_pool(name="xin", bufs=3) as xpool,
            tc.tile_pool(name="xtb", bufs=4) as xtpool,
            tc.tile_pool(name="outb", bufs=3) as opool,
            tc.tile_pool(name="pt", bufs=3, space="PSUM") as ptpool,
            tc.tile_pool(name="pz", bufs=3, space="PSUM") as pzpool,
        ):
            hm_sb = cpool.tile([HEAD_DIM, HEAD_DIM], mybir.dt.float32)
            nc.sync.dma_start(hm_sb[:], hm[:])
            id_sb = cpool.tile([HEAD_DIM, HEAD_DIM], mybir.dt.float32)
            nc.sync.dma_start(id_sb[:], ident[:])

            for i in range(n_tiles):
                x_tile = xpool.tile([P, hidden], mybir.dt.float32)
                nc.sync.dma_start(x_tile[:], x[i * P:(i + 1) * P, :])
                o_tile = opool.tile([P, hidden], mybir.dt.float32)
                for g in range(n_groups):
                    pt = ptpool.tile([P, gw], mybir.dt.float32)
                    for j in range(group_heads):
                        c = g * gw + j * HEAD_DIM
                        nc.tensor.transpose(
                            pt[:, j * HEAD_DIM:(j + 1) * HEAD_DIM],
                            x_tile[:, c:c + HEAD_DIM],
                            id_sb[:],
                        )
                    xt_sb = xtpool.tile([P, gw], mybir.dt.float32)
                    nc.vector.tensor_copy(xt_sb[:], pt[:])
                    pz = pzpool.tile([P, gw], mybir.dt.float32)
                    for j in range(group_heads):
                        nc.tensor.matmul(
                            pz[:, j * HEAD_DIM:(j + 1) * HEAD_DIM],
                            xt_sb[:, j * HEAD_DIM:(j + 1) * HEAD_DIM],
                            hm_sb[:],
                        )
                    nc.scalar.mul(o_tile[:, g * gw:(g + 1) * gw], pz[:], scale)
                nc.sync.dma_start(out[i * P:(i + 1) * P, :], o_tile[:])
    nc.finalize()
    return nc


_NC_CACHE = {}


def _get_nc(tok_per_core: int, hidden: int):
    key = (tok_per_core, hidden)
    if key not in _NC_CACHE:
        _NC_CACHE[key] = build_nc(tok_per_core, hidden)
    return _NC_CACHE[key]


def kernel(value, **_unused) -> np.ndarray:
    value = np.ascontiguousarray(np.asarray(value), dtype=np.float32)
    tokens, hidden = value.shape
    assert tokens % N_CORES == 0
    tok_per_core = tokens // N_CORES
    nc = _get_nc(tok_per_core, hidden)
    shards = np.split(value, N_CORES, axis=0)
    in_maps = [{"x": s} for s in shards]
    res = run_bass_kernel_spmd(nc, in_maps, core_ids=list(range(N_CORES)))
    return np.concatenate([r["out"] for r in res.results], axis=0)
